# revision 1
# baseline (speedup 1.0000x reference)
"""Trainium2 Bass kernel for KGAT-HAKE message passing (8 NeuronCores).

Self-contained: host-side graph partitioning + 3 SPMD launches via
run_bass_kernel_spmd.  kernel(**inputs) -> np.ndarray [100000, 240].
"""
"""KGAT-HAKE Trainium kernel: host prep + 3 SPMD launches across 8 cores.

Sharding: nodes bin-packed into 8 cores x NBLK blocks x 128 slots (by dst
in-degree).  Edges live with their dst block, padded to B per block.
Launch A: HAKE edge attention + edge softmax denominator + layer-0
aggregation + layer-0 bi-interaction dense.  Launches B, C: layers 1, 2.
Host gathers ego_l[src] between launches (pure indexing, no FLOPs).
"""
import numpy as np
import concourse.bacc as bacc
import concourse.tile as tile
import concourse.mybir as mybir
from concourse import bass
from concourse.bass_utils import run_bass_kernel_spmd
from concourse.masks import make_identity

F32 = mybir.dt.float32
I32 = mybir.dt.int32
AF = mybir.ActivationFunctionType
OP = mybir.AluOpType

N = 100000
E = 1000000
D = 64          # half width
R = 40
NCORES = 8
SLOT = 128
NPC = 12544     # nodes per core (98 * 128)
NBLK = 98
NTOT = NCORES * NPC
PI = 3.1415926235897933
GAMMA = 12.0
EMB_RANGE = (GAMMA + 2.0) / D
SIN_SCALE = PI / (2.0 * EMB_RANGE)      # sin arg = s * SIN_SCALE
WRAP_BOUND = PI / SIN_SCALE             # = 2*EMB_RANGE/... wraps s into [-b, b]
WRAP_PERIOD = 2.0 * WRAP_BOUND


# ----------------------------------------------------------------- host prep
def host_prep(inp):
    """Returns layout dict with all per-core device arrays + host mappings."""
    src = np.asarray(inp["src"]).astype(np.int64)
    dst = np.asarray(inp["dst"]).astype(np.int64)
    etype = np.asarray(inp["etype"]).astype(np.int64)
    ent = np.asarray(inp["entity_embed"], dtype=np.float32)
    rel = np.asarray(inp["rel_embed"], dtype=np.float32)

    deg = np.bincount(dst, minlength=N)
    # bin-pack nodes into 784 blocks (<=128 nodes each), balancing edge load
    nblk_tot = NCORES * NBLK
    order = np.argsort(-deg, kind="stable")
    import heapq
    heap = [(0, i, 0) for i in range(nblk_tot)]  # (load, blk, count)
    heapq.heapify(heap)
    blk_of_node = np.empty(N, np.int32)
    slot_of_node = np.empty(N, np.int32)
    for n in order:
        while True:
            load, b, cnt = heapq.heappop(heap)
            if cnt < SLOT:
                break
        blk_of_node[n] = b
        slot_of_node[n] = cnt
        heapq.heappush(heap, (load + int(deg[n]), b, cnt + 1))
    blk_load = np.zeros(nblk_tot, np.int64)
    np.add.at(blk_load, blk_of_node[dst], 1)
    # deal blocks to cores, snake by load
    bo = np.argsort(-blk_load, kind="stable")
    core_of_blk = np.empty(nblk_tot, np.int32)
    blkidx_of_blk = np.empty(nblk_tot, np.int32)
    loads = np.zeros(NCORES)
    counts = np.zeros(NCORES, np.int32)
    for b in bo:
        c = int(np.argmin(loads + (counts >= NBLK) * 1e18))
        core_of_blk[b] = c
        blkidx_of_blk[b] = counts[c]
        counts[c] += 1
        loads[c] += blk_load[b]
    B = int(np.ceil(max(1, blk_load.max()) / SLOT) * SLOT)
    K = B // SLOT

    # per-edge placement
    eb = blk_of_node[dst]                      # global block of each edge
    eorder = np.argsort(eb, kind="stable")
    eb_s = eb[eorder]
    starts = np.searchsorted(eb_s, np.arange(nblk_tot))
    pos = np.arange(E) - starts[eb_s]          # position within block
    ec = core_of_blk[eb_s]
    ebi = blkidx_of_blk[eb_s]                  # block index within core
    ej, ep = pos // SLOT, pos % SLOT           # tile j, partition p

    # per (core, blk, p, j): edge source / dsts / etype (pad -> -1)
    esrc = np.full((NCORES, NBLK, SLOT, K), -1, np.int64)
    edst_slot = np.zeros((NCORES, NBLK, SLOT, K), np.float32)
    eet = np.zeros((NCORES, NBLK, K, SLOT), np.float32)
    emask = np.zeros((NCORES, NBLK, SLOT, K), np.float32)
    es = src[eorder]
    ed = dst[eorder]
    ee = etype[eorder]
    esrc[ec, ebi, ep, ej] = es
    edst_slot[ec, ebi, ep, ej] = slot_of_node[ed]
    eet[ec, ebi, ej, ep] = ee
    emask[ec, ebi, ep, ej] = 1.0

    # node table in permuted dense layout per core: [NBLK, SLOT, 2D]
    entd = np.zeros((NCORES, NBLK, SLOT, 2 * D), np.float32)
    cc = core_of_blk[blk_of_node]
    bb = blkidx_of_blk[blk_of_node]
    ss = slot_of_node
    entd[cc, bb, ss] = ent

    # relation table [R, 3D] -> [pr | c1 | c2]
    pr = rel[:, :D]
    mr = np.abs(rel[:, D:2 * D])
    br = np.minimum(rel[:, 2 * D:], 1.0)
    br = np.maximum(br, -mr)
    relcat = np.concatenate([pr, mr + br, 1.0 - br], axis=1).astype(np.float32)

    return dict(
        B=B, K=K, esrc=esrc, edst_slot=edst_slot, eet=eet, emask=emask,
        entd=entd, relcat=relcat, cc=cc, bb=bb, ss=ss, src=src, dst=dst,
    )


def gather_stream(lay, tab_node, width):
    """tab_node: [N, width] node-space table -> [NCORES, NBLK, SLOT, K*width]."""
    esrc = lay["esrc"]
    idx = esrc.copy()
    padm = idx < 0
    idx[padm] = 0
    out = tab_node[idx]                       # [NC, NBLK, SLOT, K, width]
    out[padm] = 0.0
    return np.ascontiguousarray(
        out.reshape(NCORES, NBLK, SLOT, lay["K"] * width).astype(np.float32))


# ----------------------------------------------------------------- launch A
def build_launch_A(Bpad, K, phase_w, mod_w):
    nc = bacc.Bacc("TRN2", target_bir_lowering=False, debug=False,
                   num_devices=NCORES)
    d = {}
    d["t0"] = nc.dram_tensor("t0", [NBLK, SLOT, K * 2 * D], F32, kind="ExternalInput")
    d["h"] = nc.dram_tensor("h", [NBLK, SLOT, K * 2 * D], F32, kind="ExternalInput")
    d["dstcol"] = nc.dram_tensor("dstcol", [SLOT, NBLK * K], F32, kind="ExternalInput")
    d["mask"] = nc.dram_tensor("mask", [SLOT, NBLK * K], F32, kind="ExternalInput")
    d["ohre"] = nc.dram_tensor("ohre", [NBLK, R, K * SLOT], F32, kind="ExternalInput")
    d["entd"] = nc.dram_tensor("entd", [NBLK, SLOT, 2 * D], F32, kind="ExternalInput")
    d["relcat"] = nc.dram_tensor("relcat", [R, 3 * D], F32, kind="ExternalInput")
    d["w1t"] = nc.dram_tensor("w1t", [2 * D, D], F32, kind="ExternalInput")
    d["b1"] = nc.dram_tensor("b1", [1, D], F32, kind="ExternalInput")
    d["w2t"] = nc.dram_tensor("w2t", [2 * D, D], F32, kind="ExternalInput")
    d["b2"] = nc.dram_tensor("b2", [1, D], F32, kind="ExternalInput")
    d["ex_o"] = nc.dram_tensor("ex_o", [SLOT, NBLK * K], F32, kind="ExternalOutput")
    d["recden_o"] = nc.dram_tensor("recden_o", [SLOT, NBLK], F32, kind="ExternalOutput")
    d["ego_o"] = nc.dram_tensor("ego_o", [SLOT, NBLK * D], F32, kind="ExternalOutput")
    d["norm_o"] = nc.dram_tensor("norm_o", [SLOT, NBLK * D], F32, kind="ExternalOutput")
    nh_dram = nc.dram_tensor("nh_tmp", [NBLK, SLOT, 2 * D], F32, kind="Internal")

    TD = 2 * D  # 128

    with tile.TileContext(nc) as tc:
        with tc.tile_pool(name="pers", bufs=1) as pers:
            dstcol_sb = pers.tile([SLOT, NBLK * K], F32)
            mask_sb = pers.tile([SLOT, NBLK * K], F32)
            ex_sb = pers.tile([SLOT, NBLK * K], F32)
            den_sb = pers.tile([SLOT, NBLK], F32)
            relcat_sb = pers.tile([R, 3 * D], F32)
            iota_full_i = pers.tile([SLOT, SLOT], I32)
            iota_full = pers.tile([SLOT, SLOT], F32)
            ident = pers.tile([SLOT, SLOT], F32)
            ones_row = pers.tile([1, SLOT], F32)
            w1t_sb = pers.tile([TD, D], F32)
            w2t_sb = pers.tile([TD, D], F32)
            b1_sb = pers.tile([1, D], F32)
            b2_sb = pers.tile([1, D], F32)

            nc.sync.dma_start(out=dstcol_sb[:], in_=d["dstcol"][:, :])
            nc.sync.dma_start(out=mask_sb[:], in_=d["mask"][:, :])
            nc.sync.dma_start(out=relcat_sb[:], in_=d["relcat"][:, :])
            nc.sync.dma_start(out=w1t_sb[:], in_=d["w1t"][:, :])
            nc.sync.dma_start(out=w2t_sb[:], in_=d["w2t"][:, :])
            nc.sync.dma_start(out=b1_sb[:], in_=d["b1"][:, :])
            nc.sync.dma_start(out=b2_sb[:], in_=d["b2"][:, :])
            nc.gpsimd.iota(iota_full_i[:], pattern=[[1, SLOT]], base=0,
                           channel_multiplier=0)
            nc.vector.tensor_copy(out=iota_full[:], in_=iota_full_i[:])
            make_identity(nc, ident[:])
            nc.vector.memset(ones_row[:], 1.0)

            # ---------------- edge phase ----------------
            with tc.tile_pool(name="edge", bufs=2) as ep, \
                 tc.tile_pool(name="edge1", bufs=1) as ep1, \
                 tc.tile_pool(name="epsum", bufs=2, space="PSUM") as pp:
                for b in range(NBLK):
                    t0 = ep.tile([SLOT, K, TD], F32, tag="t0")
                    hh = ep.tile([SLOT, K, TD], F32, tag="h")
                    ohre = ep1.tile([R, K, SLOT], F32, tag="ohre")
                    nc.sync.dma_start(out=t0[:], in_=d["t0"][b])
                    nc.sync.dma_start(out=hh[:], in_=d["h"][b])
                    nc.sync.dma_start(out=ohre[:], in_=d["ohre"][b])

                    # scatter one-hot [e, m] built for all K tiles at once
                    ohem = ep1.tile([SLOT, K, SLOT], F32, tag="ohem")
                    dcol = dstcol_sb[:, b * K:(b + 1) * K]
                    nc.vector.tensor_tensor(
                        out=ohem[:],
                        in0=dcol.unsqueeze(2).to_broadcast([SLOT, K, SLOT]),
                        in1=iota_full[:].unsqueeze(1).to_broadcast([SLOT, K, SLOT]),
                        op=OP.is_equal)
                    # gather r per tile via matmul
                    rsb = ep1.tile([SLOT, K, 3 * D], F32, tag="rsb")
                    for j in range(K):
                        rp = pp.tile([SLOT, 3 * D], F32, tag="rp")
                        nc.tensor.matmul(out=rp[:], lhsT=ohre[:, j, :],
                                         rhs=relcat_sb[:], start=True, stop=True)
                        nc.scalar.copy(out=rsb[:, j, :], in_=rp[:])

                    ph = hh[:, :, 0:D]
                    mh = hh[:, :, D:TD]
                    pt = t0[:, :, 0:D]
                    mt = t0[:, :, D:TD]
                    pr_ = rsb[:, :, 0:D]
                    c1 = rsb[:, :, D:TD]
                    c2 = rsb[:, :, TD:3 * D]

                    s1 = ep.tile([SLOT, K, D], F32, tag="s1")
                    nc.vector.tensor_tensor(out=s1[:], in0=ph, in1=pr_, op=OP.add)
                    nc.vector.tensor_tensor(out=s1[:], in0=s1[:], in1=pt,
                                            op=OP.subtract)
                    nc.vector.add_range_wrap(out=s1[:], in_=s1[:], shift=0.0,
                                             bound=WRAP_BOUND, period=WRAP_PERIOD)
                    sn = ep.tile([SLOT, K, D], F32, tag="sn")
                    nc.scalar.activation(out=sn[:], in_=s1[:], func=AF.Sin,
                                         scale=SIN_SCALE)
                    phs = ep.tile([SLOT, K], F32, tag="phs")
                    nc.vector.tensor_reduce(out=phs[:], in_=sn[:],
                                            axis=mybir.AxisListType.X, op=OP.add,
                                            apply_absolute_value=True)
                    v = ep.tile([SLOT, K, D], F32, tag="v")
                    nc.vector.tensor_tensor(out=v[:], in0=mh, in1=c1, op=OP.mult)
                    nc.vector.tensor_tensor(out=sn[:], in0=mt, in1=c2, op=OP.mult)
                    nc.vector.tensor_tensor(out=v[:], in0=v[:], in1=sn[:],
                                            op=OP.subtract)
                    nc.scalar.activation(out=sn[:], in_=v[:], func=AF.Square)
                    mss = ep.tile([SLOT, K], F32, tag="mss")
                    nc.vector.tensor_reduce(out=mss[:], in_=sn[:],
                                            axis=mybir.AxisListType.X, op=OP.add)
                    # att = phase_w*phs + mod_w*sqrt(mss); ex = exp(att)*mask
                    a1 = ep.tile([SLOT, K], F32, tag="a1")
                    nc.scalar.activation(out=a1[:], in_=mss[:], func=AF.Sqrt)
                    p2 = ep.tile([SLOT, K], F32, tag="p2")
                    nc.vector.tensor_scalar_mul(out=p2[:], in0=phs[:],
                                                scalar1=float(phase_w))
                    nc.vector.scalar_tensor_tensor(
                        out=a1[:], in0=a1[:], scalar=float(mod_w), in1=p2[:],
                        op0=OP.mult, op1=OP.add)
                    exf = ep.tile([SLOT, K], F32, tag="exf")
                    nc.scalar.activation(out=exf[:], in_=a1[:], func=AF.Exp)
                    exs = ex_sb[:, b * K:(b + 1) * K]
                    nc.vector.tensor_tensor(out=exs, in0=exf[:],
                                            in1=mask_sb[:, b * K:(b + 1) * K],
                                            op=OP.mult)
                    # msg = t0 * ex  (in place)
                    nc.gpsimd.tensor_tensor(
                        out=t0[:], in0=t0[:],
                        in1=exs.unsqueeze(2).to_broadcast([SLOT, K, TD]),
                        op=OP.mult)
                    # scatter: Nh_blk += onehot^T @ msg ; den += onehot^T @ ex
                    nhp = pp.tile([SLOT, TD], F32, tag="nhp")
                    dnp = pp.tile([SLOT, 1], F32, tag="dnp")
                    for j in range(K):
                        nc.tensor.matmul(out=nhp[:], lhsT=ohem[:, j, :],
                                         rhs=t0[:, j, :], start=(j == 0),
                                         stop=(j == K - 1))
                        nc.tensor.matmul(out=dnp[:], lhsT=ohem[:, j, :],
                                         rhs=exs[:, j:j + 1], start=(j == 0),
                                         stop=(j == K - 1))
                    nhsb = ep.tile([SLOT, TD], F32, tag="nhsb")
                    nc.scalar.copy(out=nhsb[:], in_=nhp[:])
                    nc.sync.dma_start(out=nh_dram[b], in_=nhsb[:])
                    nc.scalar.copy(out=den_sb[:, b:b + 1], in_=dnp[:])

            # ---------------- dense phase ----------------
            nc.sync.dma_start(out=d["ex_o"][:, :], in_=ex_sb[:])
            recden_sb = pers.tile([SLOT, NBLK], F32)
            nc.vector.tensor_scalar_max(out=den_sb[:], in0=den_sb[:],
                                        scalar1=1e-30)
            nc.vector.reciprocal(out=recden_sb[:], in_=den_sb[:])
            nc.sync.dma_start(out=d["recden_o"][:, :], in_=recden_sb[:])

            o_sb = pers.tile([SLOT, NBLK * D], F32)
            with tc.tile_pool(name="dense", bufs=3) as dp, \
                 tc.tile_pool(name="dpsum", bufs=2, space="PSUM") as dpp:
                for b in range(NBLK):
                    entb = dp.tile([SLOT, TD], F32, tag="entb")
                    nc.sync.dma_start(out=entb[:], in_=d["entd"][b])
                    nhb = dp.tile([SLOT, TD], F32, tag="nhb")
                    nc.sync.dma_start(out=nhb[:], in_=nh_dram[b])
                    rd = recden_sb[:, b:b + 1]
                    x1 = dp.tile([SLOT, TD], F32, tag="x1")
                    x2 = dp.tile([SLOT, TD], F32, tag="x2")
                    nc.vector.scalar_tensor_tensor(out=x2[:], in0=nhb[:],
                                                   scalar=rd, in1=entb[:],
                                                   op0=OP.mult, op1=OP.mult)
                    nc.vector.scalar_tensor_tensor(out=x1[:], in0=nhb[:],
                                                   scalar=rd, in1=entb[:],
                                                   op0=OP.mult, op1=OP.add)
                    x1tp = dpp.tile([SLOT, SLOT], F32, tag="x1tp")
                    x2tp = dpp.tile([SLOT, SLOT], F32, tag="x2tp")
                    nc.tensor.transpose(out=x1tp[:], in_=x1[:], identity=ident[:])
                    nc.tensor.transpose(out=x2tp[:], in_=x2[:], identity=ident[:])
                    x1t = dp.tile([SLOT, SLOT], F32, tag="x1t")
                    x2t = dp.tile([SLOT, SLOT], F32, tag="x2t")
                    nc.scalar.copy(out=x1t[:], in_=x1tp[:])
                    nc.scalar.copy(out=x2t[:], in_=x2tp[:])
                    o1p = dpp.tile([SLOT, D], F32, tag="o1p")
                    o2p = dpp.tile([SLOT, D], F32, tag="o2p")
                    nc.tensor.matmul(out=o1p[:], lhsT=x1t[:], rhs=w1t_sb[:],
                                     start=True, stop=False)
                    nc.tensor.matmul(out=o1p[:], lhsT=ones_row[:], rhs=b1_sb[:],
                                     start=False, stop=True)
                    nc.tensor.matmul(out=o2p[:], lhsT=x2t[:], rhs=w2t_sb[:],
                                     start=True, stop=False)
                    nc.tensor.matmul(out=o2p[:], lhsT=ones_row[:], rhs=b2_sb[:],
                                     start=False, stop=True)
                    l1 = dp.tile([SLOT, D], F32, tag="l1")
                    l2 = dp.tile([SLOT, D], F32, tag="l2")
                    nc.scalar.activation(out=l1[:], in_=o1p[:], func=AF.Lrelu,
                                         alpha=0.01)
                    nc.scalar.activation(out=l2[:], in_=o2p[:], func=AF.Lrelu,
                                         alpha=0.01)
                    nc.vector.tensor_tensor(out=o_sb[:, b * D:(b + 1) * D],
                                            in0=l1[:], in1=l2[:], op=OP.add)
            nc.sync.dma_start(out=d["ego_o"][:, :], in_=o_sb[:])
            # ---------------- norm ----------------
            sq_sb = pers.tile([SLOT, NBLK * D], F32)
            nc.scalar.activation(out=sq_sb[:], in_=o_sb[:], func=AF.Square)
            ss = pers.tile([SLOT, NBLK], F32)
            nc.vector.tensor_reduce(
                out=ss[:], in_=sq_sb[:].rearrange("p (b dd) -> p b dd", dd=D),
                axis=mybir.AxisListType.X, op=OP.add)
            nc.scalar.activation(out=ss[:], in_=ss[:], func=AF.Sqrt)
            nc.vector.tensor_scalar_max(out=ss[:], in0=ss[:], scalar1=1e-12)
            rs = pers.tile([SLOT, NBLK], F32)
            nc.vector.reciprocal(out=rs[:], in_=ss[:])
            nc.vector.tensor_tensor(
                out=sq_sb[:].rearrange("p (b dd) -> p b dd", dd=D),
                in0=o_sb[:].rearrange("p (b dd) -> p b dd", dd=D),
                in1=rs[:].unsqueeze(2).to_broadcast([SLOT, NBLK, D]),
                op=OP.mult)
            nc.sync.dma_start(out=d["norm_o"][:, :], in_=sq_sb[:])

    nc.compile()
    return nc


# ----------------------------------------------------------------- launch B/C
def build_launch_BC(Bpad, K, din, dout):
    nc = bacc.Bacc("TRN2", target_bir_lowering=False, debug=False,
                   num_devices=NCORES)
    d = {}
    d["t"] = nc.dram_tensor("t", [NBLK, SLOT, K * din], F32, kind="ExternalInput")
    d["dstcol"] = nc.dram_tensor("dstcol", [SLOT, NBLK * K], F32, kind="ExternalInput")
    d["ex_i"] = nc.dram_tensor("ex_i", [SLOT, NBLK * K], F32, kind="ExternalInput")
    d["recden_i"] = nc.dram_tensor("recden_i", [SLOT, NBLK], F32, kind="ExternalInput")
    d["egod"] = nc.dram_tensor("egod", [SLOT, NBLK * din], F32, kind="ExternalInput")
    d["w1t"] = nc.dram_tensor("w1t", [din, dout], F32, kind="ExternalInput")
    d["b1"] = nc.dram_tensor("b1", [1, dout], F32, kind="ExternalInput")
    d["w2t"] = nc.dram_tensor("w2t", [din, dout], F32, kind="ExternalInput")
    d["b2"] = nc.dram_tensor("b2", [1, dout], F32, kind="ExternalInput")
    d["ego_o"] = nc.dram_tensor("ego_o", [SLOT, NBLK * dout], F32, kind="ExternalOutput")
    d["norm_o"] = nc.dram_tensor("norm_o", [SLOT, NBLK * dout], F32, kind="ExternalOutput")

    with tile.TileContext(nc) as tc:
        with tc.tile_pool(name="pers", bufs=1) as pers:
            dstcol_sb = pers.tile([SLOT, NBLK * K], F32)
            ex_sb = pers.tile([SLOT, NBLK * K], F32)
            recden_sb = pers.tile([SLOT, NBLK], F32)
            egod_sb = pers.tile([SLOT, NBLK * din], F32)
            nh_sb = pers.tile([SLOT, NBLK * din], F32)
            o_sb = pers.tile([SLOT, NBLK * dout], F32)
            iota_full_i = pers.tile([SLOT, SLOT], I32)
            iota_full = pers.tile([SLOT, SLOT], F32)
            ident = pers.tile([SLOT, SLOT], F32)
            ones_row = pers.tile([1, SLOT], F32)
            w1t_sb = pers.tile([din, dout], F32)
            w2t_sb = pers.tile([din, dout], F32)
            b1_sb = pers.tile([1, dout], F32)
            b2_sb = pers.tile([1, dout], F32)

            nc.sync.dma_start(out=dstcol_sb[:], in_=d["dstcol"][:, :])
            nc.sync.dma_start(out=ex_sb[:], in_=d["ex_i"][:, :])
            nc.sync.dma_start(out=recden_sb[:], in_=d["recden_i"][:, :])
            nc.sync.dma_start(out=egod_sb[:], in_=d["egod"][:, :])
            nc.sync.dma_start(out=w1t_sb[:], in_=d["w1t"][:, :])
            nc.sync.dma_start(out=w2t_sb[:], in_=d["w2t"][:, :])
            nc.sync.dma_start(out=b1_sb[:], in_=d["b1"][:, :])
            nc.sync.dma_start(out=b2_sb[:], in_=d["b2"][:, :])
            nc.gpsimd.iota(iota_full_i[:], pattern=[[1, SLOT]], base=0,
                           channel_multiplier=0)
            nc.vector.tensor_copy(out=iota_full[:], in_=iota_full_i[:])
            make_identity(nc, ident[:])
            nc.vector.memset(ones_row[:], 1.0)

            with tc.tile_pool(name="edge", bufs=2) as ep, \
                 tc.tile_pool(name="epsum", bufs=2, space="PSUM") as pp:
                for b in range(NBLK):
                    t = ep.tile([SLOT, K, din], F32, tag="t")
                    nc.sync.dma_start(out=t[:], in_=d["t"][b])
                    ohem = ep.tile([SLOT, K, SLOT], F32, tag="ohem")
                    dcol = dstcol_sb[:, b * K:(b + 1) * K]
                    nc.vector.tensor_tensor(
                        out=ohem[:],
                        in0=dcol.unsqueeze(2).to_broadcast([SLOT, K, SLOT]),
                        in1=iota_full[:].unsqueeze(1).to_broadcast([SLOT, K, SLOT]),
                        op=OP.is_equal)
                    exs = ex_sb[:, b * K:(b + 1) * K]
                    nc.gpsimd.tensor_tensor(
                        out=t[:], in0=t[:],
                        in1=exs.unsqueeze(2).to_broadcast([SLOT, K, din]),
                        op=OP.mult)
                    nhp = pp.tile([SLOT, din], F32, tag="nhp")
                    for j in range(K):
                        nc.tensor.matmul(out=nhp[:], lhsT=ohem[:, j, :],
                                         rhs=t[:, j, :], start=(j == 0),
                                         stop=(j == K - 1))
                    nc.scalar.copy(out=nh_sb[:, b * din:(b + 1) * din], in_=nhp[:])

            with tc.tile_pool(name="dense", bufs=3) as dp, \
                 tc.tile_pool(name="dpsum", bufs=2, space="PSUM") as dpp:
                for b in range(NBLK):
                    rd = recden_sb[:, b:b + 1]
                    egb = egod_sb[:, b * din:(b + 1) * din]
                    nhb = nh_sb[:, b * din:(b + 1) * din]
                    x1 = dp.tile([SLOT, din], F32, tag="x1")
                    x2 = dp.tile([SLOT, din], F32, tag="x2")
                    nc.vector.scalar_tensor_tensor(out=x2[:], in0=nhb, scalar=rd,
                                                   in1=egb, op0=OP.mult,
                                                   op1=OP.mult)
                    nc.vector.scalar_tensor_tensor(out=x1[:], in0=nhb, scalar=rd,
                                                   in1=egb, op0=OP.mult,
                                                   op1=OP.add)
                    x1tp = dpp.tile([din, SLOT], F32, tag="x1tp")
                    x2tp = dpp.tile([din, SLOT], F32, tag="x2tp")
                    nc.tensor.transpose(out=x1tp[:], in_=x1[:], identity=ident[:])
                    nc.tensor.transpose(out=x2tp[:], in_=x2[:], identity=ident[:])
                    x1t = dp.tile([din, SLOT], F32, tag="x1t")
                    x2t = dp.tile([din, SLOT], F32, tag="x2t")
                    nc.scalar.copy(out=x1t[:], in_=x1tp[:])
                    nc.scalar.copy(out=x2t[:], in_=x2tp[:])
                    o1p = dpp.tile([SLOT, dout], F32, tag="o1p")
                    o2p = dpp.tile([SLOT, dout], F32, tag="o2p")
                    nc.tensor.matmul(out=o1p[:], lhsT=x1t[:], rhs=w1t_sb[:],
                                     start=True, stop=False)
                    nc.tensor.matmul(out=o1p[:], lhsT=ones_row[:], rhs=b1_sb[:],
                                     start=False, stop=True)
                    nc.tensor.matmul(out=o2p[:], lhsT=x2t[:], rhs=w2t_sb[:],
                                     start=True, stop=False)
                    nc.tensor.matmul(out=o2p[:], lhsT=ones_row[:], rhs=b2_sb[:],
                                     start=False, stop=True)
                    l1 = dp.tile([SLOT, dout], F32, tag="l1")
                    l2 = dp.tile([SLOT, dout], F32, tag="l2")
                    nc.scalar.activation(out=l1[:], in_=o1p[:], func=AF.Lrelu,
                                         alpha=0.01)
                    nc.scalar.activation(out=l2[:], in_=o2p[:], func=AF.Lrelu,
                                         alpha=0.01)
                    nc.vector.tensor_tensor(out=o_sb[:, b * dout:(b + 1) * dout],
                                            in0=l1[:], in1=l2[:], op=OP.add)
            nc.sync.dma_start(out=d["ego_o"][:, :], in_=o_sb[:])
            sq_sb = pers.tile([SLOT, NBLK * dout], F32)
            nc.scalar.activation(out=sq_sb[:], in_=o_sb[:], func=AF.Square)
            ss = pers.tile([SLOT, NBLK], F32)
            nc.vector.tensor_reduce(
                out=ss[:], in_=sq_sb[:].rearrange("p (b dd) -> p b dd", dd=dout),
                axis=mybir.AxisListType.X, op=OP.add)
            nc.scalar.activation(out=ss[:], in_=ss[:], func=AF.Sqrt)
            nc.vector.tensor_scalar_max(out=ss[:], in0=ss[:], scalar1=1e-12)
            rs = pers.tile([SLOT, NBLK], F32)
            nc.vector.reciprocal(out=rs[:], in_=ss[:])
            nc.vector.tensor_tensor(
                out=sq_sb[:].rearrange("p (b dd) -> p b dd", dd=dout),
                in0=o_sb[:].rearrange("p (b dd) -> p b dd", dd=dout),
                in1=rs[:].unsqueeze(2).to_broadcast([SLOT, NBLK, dout]),
                op=OP.mult)
            nc.sync.dma_start(out=d["norm_o"][:, :], in_=sq_sb[:])

    nc.compile()
    return nc


# ----------------------------------------------------------------- driver
def node_table_from_out(lay, out_pc, width):
    """out_pc: list per core of [SLOT, NBLK*width] -> node-space [N, width]."""
    tab = np.empty((N, width), np.float32)
    cc, bb, ss = lay["cc"], lay["bb"], lay["ss"]
    stk = np.stack(out_pc)  # [NC, SLOT, NBLK*width]
    stk = stk.reshape(NCORES, SLOT, NBLK, width)
    tab[:] = stk[cc, ss, bb]
    return tab


def run(inp, trace=False, verbose=True):
    import time
    lay = host_prep(inp)
    B_, K = lay["B"], lay["K"]
    if verbose:
        print(f"host_prep done: B={B_} K={K}")
    phase_w = float(np.asarray(inp["phase_w"]).reshape(-1)[0])
    mod_w = float(np.asarray(inp["mod_w"]).reshape(-1)[0])
    ent = np.asarray(inp["entity_embed"], dtype=np.float32)

    t0s = gather_stream(lay, ent, 2 * D)
    # h stream: entity[dst]
    hs = hstream(lay, ent)
    dstcol = np.ascontiguousarray(
        lay["edst_slot"].transpose(0, 2, 1, 3).reshape(NCORES, SLOT, NBLK * K))
    maskk = np.ascontiguousarray(
        lay["emask"].transpose(0, 2, 1, 3).reshape(NCORES, SLOT, NBLK * K))
    ohre = np.ascontiguousarray(
        (lay["eet"].reshape(NCORES, NBLK, 1, K * SLOT)
         == np.arange(R, dtype=np.float32).reshape(1, 1, R, 1))
        .astype(np.float32))

    exec_ns = 0
    t0c = time.time()
    ncA = build_launch_A(B_, K, phase_w, mod_w)
    if verbose:
        print(f"A compiled in {time.time()-t0c:.1f}s")
    in_maps = []
    for c in range(NCORES):
        in_maps.append(dict(
            t0=t0s[c], h=hs[c], dstcol=dstcol[c], mask=maskk[c], ohre=ohre[c],
            entd=lay["entd"][c], relcat=lay["relcat"],
            w1t=np.ascontiguousarray(inp["W1_0"].T, dtype=np.float32),
            b1=np.asarray(inp["b1_0"], np.float32).reshape(1, -1),
            w2t=np.ascontiguousarray(inp["W2_0"].T, dtype=np.float32),
            b2=np.asarray(inp["b2_0"], np.float32).reshape(1, -1),
        ))
    t0c = time.time()
    resA = run_bass_kernel_spmd(ncA, in_maps, core_ids=list(range(NCORES)),
                                trace=trace)
    if verbose:
        print(f"A ran in {time.time()-t0c:.1f}s exec_ns={resA.exec_time_ns}")
    if resA.exec_time_ns:
        exec_ns += resA.exec_time_ns

    ego1 = node_table_from_out(lay, [r["ego_o"] for r in resA.results], D)
    ex_pc = [r["ex_o"] for r in resA.results]
    recden_pc = [r["recden_o"] for r in resA.results]
    norm1 = node_table_from_out(lay, [r["norm_o"] for r in resA.results], D)

    # ---- launch B
    t1s = gather_stream(lay, ego1, D)
    t0c = time.time()
    ncB = build_launch_BC(B_, K, D, 32)
    if verbose:
        print(f"B compiled in {time.time()-t0c:.1f}s")
    in_maps = []
    for c in range(NCORES):
        in_maps.append(dict(
            t=t1s[c], dstcol=dstcol[c], ex_i=ex_pc[c], recden_i=recden_pc[c],
            egod=resA.results[c]["ego_o"],
            w1t=np.ascontiguousarray(inp["W1_1"].T, dtype=np.float32),
            b1=np.asarray(inp["b1_1"], np.float32).reshape(1, -1),
            w2t=np.ascontiguousarray(inp["W2_1"].T, dtype=np.float32),
            b2=np.asarray(inp["b2_1"], np.float32).reshape(1, -1),
        ))
    t0c = time.time()
    resB = run_bass_kernel_spmd(ncB, in_maps, core_ids=list(range(NCORES)),
                                trace=trace)
    if verbose:
        print(f"B ran in {time.time()-t0c:.1f}s exec_ns={resB.exec_time_ns}")
    if resB.exec_time_ns:
        exec_ns += resB.exec_time_ns
    ego2 = node_table_from_out(lay, [r["ego_o"] for r in resB.results], 32)
    norm2 = node_table_from_out(lay, [r["norm_o"] for r in resB.results], 32)

    # ---- launch C
    t2s = gather_stream(lay, ego2, 32)
    t0c = time.time()
    ncC = build_launch_BC(B_, K, 32, 16)
    if verbose:
        print(f"C compiled in {time.time()-t0c:.1f}s")
    in_maps = []
    for c in range(NCORES):
        in_maps.append(dict(
            t=t2s[c], dstcol=dstcol[c], ex_i=ex_pc[c], recden_i=recden_pc[c],
            egod=resB.results[c]["ego_o"],
            w1t=np.ascontiguousarray(inp["W1_2"].T, dtype=np.float32),
            b1=np.asarray(inp["b1_2"], np.float32).reshape(1, -1),
            w2t=np.ascontiguousarray(inp["W1_2b"].T, dtype=np.float32),
            b2=np.asarray(inp["b2_2"], np.float32).reshape(1, -1),
        ))
    t0c = time.time()
    resC = run_bass_kernel_spmd(ncC, in_maps, core_ids=list(range(NCORES)),
                                trace=trace)
    if verbose:
        print(f"C ran in {time.time()-t0c:.1f}s exec_ns={resC.exec_time_ns}")
    if resC.exec_time_ns:
        exec_ns += resC.exec_time_ns
    norm3 = node_table_from_out(lay, [r["norm_o"] for r in resC.results], 16)

    out = np.concatenate([ent, norm1, norm2, norm3], axis=1)
    return out, exec_ns


def hstream(lay, ent):
    """h = entity[dst] stream in the same [NBLK, SLOT, K*2D] layout."""
    K = lay["K"]
    # rebuild dst node ids per slot position from esrc-like bookkeeping:
    # we stored edst_slot (slot only); reconstruct via block node table:
    # simpler: entd[c, b, slot] is entity of that node -> h = entd[c,b,slot(dst)]
    sl = lay["edst_slot"].astype(np.int64)          # [NC, NBLK, SLOT, K]
    entd = lay["entd"]                              # [NC, NBLK, SLOT, 2D]
    cidx = np.arange(NCORES)[:, None, None, None]
    bidx = np.arange(NBLK)[None, :, None, None]
    h = entd[cidx, bidx, sl]                        # [NC, NBLK, SLOT, K, 2D]
    h = h * lay["emask"][..., None]
    return np.ascontiguousarray(
        h.reshape(NCORES, NBLK, SLOT, K * 2 * D).astype(np.float32))


# ----------------------------------------------------------------- entry
TRACE = False
LAST_EXEC_NS = None


def _install_ntff_hook():
    import sys, types, contextlib
    if "antenv.axon_hooks" in sys.modules:
        return True
    try:
        mod = types.ModuleType("antenv.axon_hooks")
        mod._hook = None
        mod.set_axon_ntff_profile_hook = lambda h: setattr(mod, "_hook", h)
        mod.get_axon_ntff_profile_hook = lambda: mod._hook
        import antenv
        sys.modules["antenv.axon_hooks"] = mod
        antenv.axon_hooks = mod
        from trn_agent_boot.trn_boot import _ntff_profile_via_ctypes
        h = _ntff_profile_via_ctypes("/opt/axon/libaxon_pjrt.so")
        if h is None:
            return False
        mod._hook = h
        return True
    except Exception:
        return False


def kernel(**inputs):
    global LAST_EXEC_NS
    trace = TRACE and _install_ntff_hook()
    out, exec_ns = run(inputs, trace=trace, verbose=False)
    LAST_EXEC_NS = exec_ns
    return out
